# revision 1
# baseline (speedup 1.0000x reference)
# Trainium2 Bass kernel for nn_ContextualAttention_5669356832394.
#
# Reference computation (per sample):
#   - downsample f,b,mask by 2 (nearest)
#   - w  = 3x3 patches of b_s (L=1024 patches), normalized
#   - Y[p, q] = <w_p, patch_q(f_s)>  (correlation GEMM, K=1152)
#   - two "fuse" convs: diagonal 3-tap sums over flattened (Lb, Lf) with a
#     digit-swap permutation between passes
#   - softmax over p (the 1024 b-patches) per f-site, scaled by mask patch mean
#   - deconv (conv_transpose stride 2, kernel 4) with raw 4x4 patches of b
#
# Sharding: 8 cores = 4 samples x 2 halves of the f-site grid (fi<16 / fi>=16).
# Each core computes the full Y/fuse for its sample (cheap, keeps the program
# SPMD-identical), then softmax/deconv only for its own 512 f-sites, selected
# with a single tc.If on partition_id. Host sums the 2-row overlap of the two
# output bands per sample.
#
# On-chip layout: Y is stored transposed, [q on partitions (8 tiles of 128),
# p on free axis], so softmax is a free-axis reduction and both GEMM1 operands
# are natural c-on-partition strided views of padded SBUF arrays.

import os
import numpy as np

GEMM1_MODE = os.environ.get("CA_GEMM1_MODE", "f32r")  # f32r | f32 | bf16x3

_NC = None


def _build_nc(mode=GEMM1_MODE):
    import concourse.bass as bass
    import concourse.mybir as mybir
    import concourse.tile as tile
    from concourse import bacc
    from concourse.masks import make_identity
    from contextlib import ExitStack

    dt = mybir.dt
    AF = mybir.ActivationFunctionType
    OP = mybir.AluOpType
    ds = bass.ds

    nc = bacc.Bacc(
        "TRN2",
        target_bir_lowering=False,
        debug=False,
        num_devices=8,
        enable_partition_id=True,
    )

    fb = nc.dram_tensor("fb", [128, 64, 64], dt.float32, kind="ExternalInput").ap()
    bb = nc.dram_tensor("bb", [128, 64, 64], dt.float32, kind="ExternalInput").ap()
    mb = nc.dram_tensor("mb", [1, 64, 64], dt.float32, kind="ExternalInput").ap()
    out_d = nc.dram_tensor("out", [128, 36, 66], dt.float32, kind="ExternalOutput").ap()
    msm_d = nc.dram_tensor("msm_scr", [1, 1024], dt.float32, kind="Internal").ap()
    inv_d = nc.dram_tensor("inv_scr", [1, 1024], dt.float32, kind="Internal").ap()

    with tile.TileContext(nc) as tc, ExitStack() as ctx:
        sing = ctx.enter_context(tc.tile_pool(name="sing", bufs=1))
        scr = ctx.enter_context(tc.tile_pool(name="scr", bufs=2))
        ya_p = ctx.enter_context(tc.tile_pool(name="ya", bufs=2))
        yb_p = ctx.enter_context(tc.tile_pool(name="yb", bufs=2))
        za_p = ctx.enter_context(tc.tile_pool(name="za", bufs=2))
        zb_p = ctx.enter_context(tc.tile_pool(name="zb", bufs=2))
        rwq_p = ctx.enter_context(tc.tile_pool(name="rwq", bufs=2))
        sm = ctx.enter_context(tc.tile_pool(name="sm", bufs=4))
        pj = ctx.enter_context(tc.tile_pool(name="pj", bufs=2, space="PSUM"))
        pn = ctx.enter_context(tc.tile_pool(name="pn", bufs=1, space="PSUM"))
        pt = ctx.enter_context(tc.tile_pool(name="pt", bufs=2, space="PSUM"))
        po = ctx.enter_context(tc.tile_pool(name="po", bufs=2, space="PSUM"))

        f32, bf16 = dt.float32, dt.bfloat16

        # ---------- persistent SBUF tensors ----------
        m2 = sing.tile([128, 1088], f32, tag="m2")      # mask patch mean /9 (windowed)
        outb = sing.tile([128, 2376], f32, tag="outb")  # output band [c, 36*66]
        inv_rep = sing.tile([128, 1024], f32, tag="invrep")
        mmq4 = sing.tile([128, 8], f32, tag="mmq4")
        ones_c = sing.tile([128, 1], f32, tag="ones")
        ident = sing.tile([128, 128], bf16, tag="ident")
        # pitch-32 window planes: make every matmul weights AP a flat span.
        # For f32r mode the planes are stored as float32r (pre-rounded by the
        # producing DVE copy) since walrus requires f32r operands be produced
        # rounded.
        g1dt = dt.float32r if mode == "f32r" else f32
        fq = [sing.tile([128, 1088], g1dt, tag=f"fq{k}", name=f"fq{k}")
              for k in range(3)]  # fq[kw][c, r*32+u] = fs_pad[c, r, u+kw]
        bq = [sing.tile([128, 1088], g1dt, tag=f"bq{k}", name=f"bq{k}")
              for k in range(3)]  # bq[kw][c, r*32+u] = bs_pad[c, r, u+kw]
        # qpl[(alpha, beta, kw2)][c, r*32+u] = b_pad2[c, 2r+alpha, 2(u+kw2)+beta]
        qpl = {}
        for al in range(2):
            for be in range(2):
                for kw2 in range(2):
                    qpl[(al, be, kw2)] = sing.tile(
                        [128, 1056], bf16, tag=f"q{al}{be}{kw2}",
                        name=f"q{al}{be}{kw2}")

        y = [sing.tile([128, 1024], f32, tag=f"y{t}", name=f"y{t}") for t in range(8)]
        z1 = [sing.tile([128, 1024], f32, tag=f"z1{t}", name=f"z1{t}") for t in range(8)]
        # z2 reuses the y slots (y is dead before any z2 write: all fuse1 reads
        # of y precede z1[7], and every z2 write depends on z1[7] via carries)
        z2 = [sing.tile([128, 1024], f32, tag=f"y{t}", name=f"z2{t}") for t in range(8)]

        sync = nc.sync
        gps = nc.gpsimd
        V = nc.vector
        S = nc.scalar

        early_cm = tc.tile_pool(name="early", bufs=1)
        early = early_cm.__enter__()
        fs = early.tile([128, 1156], f32, tag="fs")     # padded f_s [c, 34*34]
        bs = early.tile([128, 1160], f32, tag="bs")     # padded b_s [c, 34*34]
        bfb = early.tile([128, 4356], bf16, tag="bfb")  # padded b [c, 66*66] bf16
        norm2 = early.tile([1, 1024], f32, tag="norm2")
        inv_w = early.tile([1, 1024], f32, tag="invw")
        inv_w2 = early.tile([1, 1024], f32, tag="invw2")

        # ---------- input loads ----------
        msr = scr.tile([128, 1156], f32, tag="scr", name="msr")
        gps.memset(outb[:, :], 0.0)
        gps.memset(fs[:, :], 0.0)
        gps.memset(bs[:, :], 0.0)
        gps.memset(bfb[:, :], 0.0)
        gps.memset(msr[:, :], 0.0)

        fsr = fs[:].rearrange("c (h w) -> c h w", w=34)
        bsr = bs[:, 0:1156].rearrange("c (h w) -> c h w", w=34)
        bfr = bfb[:].rearrange("c (h w) -> c h w", w=66)
        msv = msr[:].rearrange("c (h w) -> c h w", w=34)

        # DMA even rows at full width (contiguous innermost), then downsample
        # columns on-chip. 16-row chunks keep the c-stride from merging with
        # the row dim, so both DMA APs stay <=3 dims.
        for src_d, dst in ((fb, fsr), (bb, bsr)):
            for rr in range(2):
                stg = scr.tile([128, 1024], f32, tag="scr", name=f"stg{rr}")
                stgv = stg[:].rearrange("c (h w) -> c h w", w=64)
                sync.dma_start(out=stgv[:, :, :], in_=src_d[:, ds(32 * rr, 16, 2), :])
                V.tensor_copy(dst[:, 1 + 16 * rr:17 + 16 * rr, 1:33],
                              stgv[:, :, ds(0, 32, 2)])
        # full-res b, cast to bf16 during DMA (SWDGE)
        gps.dma_start(out=bfr[:, 1:65, 1:65], in_=bb[:, :, :])
        # mask_s: even rows to partition 0, downsample cols, then broadcast
        msm = early.tile([1, 1024], f32, tag="msm")
        for rr in range(2):
            mstg = scr.tile([1, 1024], f32, tag="scr", name=f"mstg{rr}")
            mstgv = mstg[0:1].rearrange("c (h w) -> c h w", w=64)
            sync.dma_start(out=mstgv[:, :, :], in_=mb[:, ds(32 * rr, 16, 2), :])
            V.tensor_copy(
                msm[0:1].rearrange("c (h w) -> c h w", w=32)[:, 16 * rr:16 * rr + 16, :],
                mstgv[:, :, ds(0, 32, 2)])
        sync.dma_start(out=msm_d[:, :], in_=msm[:, :])
        msm_bc = bass.AP(
            tensor=msm_d.tensor, offset=msm_d.offset,
            ap=[[0, 128], [32, 32], [1, 32]],
        )
        gps.dma_start(out=msv[:, 1:33, 1:33], in_=msm_bc)

        make_identity(nc, ident[:])
        V.memset(ones_c[:, :], 1.0)

        # build the window planes
        for kw in range(3):
            V.tensor_copy(fq[kw][:].rearrange("c (r u) -> c r u", u=32),
                          fsr[:, 0:34, kw:kw + 32])
            V.tensor_copy(bq[kw][:].rearrange("c (r u) -> c r u", u=32),
                          bsr[:, 0:34, kw:kw + 32])
        for al in range(2):
            for be in range(2):
                for kw2 in range(2):
                    S.copy(qpl[(al, be, kw2)][:].rearrange("c (r u) -> c r u", u=32),
                           bfr[:, ds(al, 33, 2), ds(2 * kw2 + be, 32, 2)])

        # ---------- mask patch means (x 1/9) ----------
        m1 = scr.tile([128, 1154], f32, tag="scr")
        V.tensor_tensor(m1[:, :], msr[:, 0:1154], msr[:, 1:1155], op=OP.add)
        V.tensor_tensor(m1[:, :], m1[:, :], msr[:, 2:1156], op=OP.add)
        V.tensor_tensor(m2[:, 0:1086], m1[:, 0:1086], m1[:, 34:1120], op=OP.add)
        V.tensor_tensor(m2[:, 0:1086], m2[:, 0:1086], m1[:, 68:1154], op=OP.add)
        V.tensor_scalar_mul(m2[:, 0:1086], m2[:, 0:1086], 1.0 / 9.0)
        m2v = m2[:].rearrange("c (a b) -> c a b", b=34)[:, :, 0:32]  # [128,32,32] mm

        # mmq4[part, pp] = mm[128*pp + part] * 0.25  (for the A^T scaling;
        # folds the deconv /4 and the 2nd mask multiply)
        mmc = early.tile([1, 1024], f32, tag="mmc")
        V.tensor_copy(mmc[0:1].rearrange("c (a b) -> c a b", b=32),
                      m2v[0:1, :, :])
        ident1 = sing.tile([1, 1], f32, tag="ident1")
        V.memset(ident1[:, :], 1.0)
        psq = pn.tile([128, 8], f32, tag="pnq")
        for pp in range(8):
            nc.tensor.matmul(psq[:, pp:pp + 1], mmc[0:1, 128 * pp:128 * pp + 128],
                             ident1[:, :], is_transpose=True,
                             start=(pp == 0), stop=(pp == 7))
        V.tensor_scalar_mul(mmq4[:, :], psq[:, :], 0.25)

        # ---------- patch norms -> inv_w, inv_rep ----------
        sq = scr.tile([128, 1156], f32, tag="scr")
        S.activation(sq[:, :], bs[:, 0:1156], AF.Square)
        r1 = scr.tile([128, 1154], f32, tag="scr")
        V.tensor_tensor(r1[:, :], sq[:, 0:1154], sq[:, 1:1155], op=OP.add)
        V.tensor_tensor(r1[:, :], r1[:, :], sq[:, 2:1156], op=OP.add)
        r2 = scr.tile([128, 1088], f32, tag="scr")
        V.memset(r2[:, 1086:1088], 0.0)
        V.tensor_tensor(r2[:, 0:1086], r1[:, 0:1086], r1[:, 34:1120], op=OP.add)
        V.tensor_tensor(r2[:, 0:1086], r2[:, 0:1086], r1[:, 68:1154], op=OP.add)
        for n in range(4):
            psn = pn.tile([1, 272], f32, tag="pn")
            nc.tensor.matmul(
                psn[:, :], ones_c[:, :], r2[:, 272 * n:272 * n + 272],
                start=True, stop=True,
            )
            # norm2 = patch_sumsq + 1152e-4 (drop the 2 pitch-pad columns/row)
            psnv = psn[:].rearrange("c (a b) -> c a b", b=34)[:, :, 0:32]
            S.activation(norm2[0:1].rearrange("c (a b) -> c a b", b=32)[:, 8 * n:8 * n + 8, :],
                         psnv, AF.Copy, bias=0.1152)
        S.activation(inv_w[:, :], norm2[:, :], AF.Sqrt)
        V.reciprocal(inv_w[:, :], inv_w[:, :])
        # one Newton step: r' = r * (1.5 - 0.5 * x * r^2) cleans up Sqrt's ULPs
        V.tensor_tensor(inv_w2[:, :], inv_w[:, :], inv_w[:, :], op=OP.mult)
        V.tensor_tensor(inv_w2[:, :], inv_w2[:, :], norm2[:, :], op=OP.mult)
        S.activation(inv_w2[:, :], inv_w2[:, :], AF.Copy, bias=1.5, scale=-0.5)
        V.tensor_tensor(inv_w2[:, :], inv_w[:, :], inv_w2[:, :], op=OP.mult)
        sync.dma_start(out=inv_d[:, :], in_=inv_w2[:, :])
        inv_src = bass.AP(
            tensor=inv_d.tensor, offset=inv_d.offset,
            ap=[[0, 128], [1, 1024]],
        )
        gps.dma_start(out=inv_rep[:, :], in_=inv_src)

        # free the early-phase SBUF (fs, bfb, norm scratch); the post-softmax
        # pools below reuse that space
        early_cm.__exit__(None, None, None)
        soft = ctx.enter_context(tc.tile_pool(name="soft", bufs=2))
        expool = ctx.enter_context(tc.tile_pool(name="expool", bufs=2))
        late = ctx.enter_context(tc.tile_pool(name="late", bufs=1))

        # ---------- GEMM1: Y^T[q, p] = sum_k F[k,q] B[k,p] * inv_w[p] ----------
        if mode == "bf16x3":
            fq_hi = [sing.tile([128, 1088], bf16, tag=f"fqh{k}", name=f"fqh{k}")
                     for k in range(3)]
            fq_lo = [sing.tile([128, 1088], bf16, tag=f"fql{k}", name=f"fql{k}")
                     for k in range(3)]
            bq_hi = [sing.tile([128, 1088], bf16, tag=f"bqh{k}", name=f"bqh{k}")
                     for k in range(3)]
            bq_lo = [sing.tile([128, 1088], bf16, tag=f"bql{k}", name=f"bql{k}")
                     for k in range(3)]
            for k in range(3):
                tmp = scr.tile([128, 1088], f32, tag="scr", name=f"tmpf{k}")
                V.tensor_copy(fq_hi[k][:, :], fq[k][:, :])
                V.tensor_tensor(tmp[:, :], fq[k][:, :], fq_hi[k][:, :],
                                op=OP.subtract)
                V.tensor_copy(fq_lo[k][:, :], tmp[:, :])
                tmpb = scr.tile([128, 1088], f32, tag="scr", name=f"tmpb{k}")
                V.tensor_copy(bq_hi[k][:, :], bq[k][:, :])
                V.tensor_tensor(tmpb[:, :], bq[k][:, :], bq_hi[k][:, :],
                                op=OP.subtract)
                V.tensor_copy(bq_lo[k][:, :], tmpb[:, :])

        def g1_ops(kh, kw, t, n):
            foff = (4 * t + kh) * 32
            boff = (8 * n + kh) * 32 + 0
            if mode == "bf16x3":
                fh = fq_hi[kw][:, foff:foff + 128]
                fl = fq_lo[kw][:, foff:foff + 128]
                bh = bq_hi[kw][:, boff:boff + 256]
                bl = bq_lo[kw][:, boff:boff + 256]
                return [(fh, bh), (fh, bl), (fl, bh)]
            return [(fq[kw][:, foff:foff + 128], bq[kw][:, boff:boff + 256])]

        for t in range(8):
            for n in range(4):
                psy = pj.tile([128, 256], f32, tag="pj")
                ops = []
                for kh in range(3):
                    for kw in range(3):
                        ops.extend(g1_ops(kh, kw, t, n))
                for i, (lv, rv) in enumerate(ops):
                    nc.tensor.matmul(
                        psy[:, :], lv, rv,
                        start=(i == 0), stop=(i == len(ops) - 1),
                    )
                V.tensor_tensor(
                    y[t][:, 256 * n:256 * n + 256], psy[:, :],
                    inv_rep[:, 256 * n:256 * n + 256], op=OP.mult,
                )

        # ---------- fuse pass 1: Z1[q,p] = Y[q-1,p-1] + Y[q,p] + Y[q+1,p+1] ----------
        for t in range(8):
            ya = ya_p.tile([128, 1025], f32, tag="ya")
            V.memset(ya[:, 0:1], 0.0)
            if t == 0:
                V.memset(ya[0:1, 1:1025], 0.0)
            else:
                sync.dma_start(out=ya[0:1, 1:1025], in_=y[t - 1][127:128, :])
            sync.dma_start(out=ya[1:128, 1:1025], in_=y[t][0:127, :])

            yb = yb_p.tile([128, 1025], f32, tag="yb")
            V.memset(yb[:, 1024:1025], 0.0)
            if t == 7:
                V.memset(yb[96:128, 0:1024], 0.0)
            else:
                sync.dma_start(out=yb[127:128, 0:1024], in_=y[t + 1][0:1, :])
            sync.dma_start(out=yb[0:127, 0:1024], in_=y[t][1:128, :])

            V.tensor_tensor(z1[t][:, :], y[t][:, :], ya[:, 0:1024], op=OP.add)
            V.tensor_tensor(z1[t][:, :], z1[t][:, :], yb[:, 1:1025], op=OP.add)

        # ---------- fuse pass 2 (digit-swapped): shifts of +-32 with carries ----------
        for t in range(8):
            za = za_p.tile([128, 1088], f32, tag="za")
            V.memset(za[:, 0:32], 0.0)
            V.memset(za[:, 1056:1088], 0.0)
            sync.dma_start(out=za[32:128, 32:1056], in_=z1[t][0:96, :])
            if t == 0:
                V.memset(za[0:1, 32:1056], 0.0)
                sync.dma_start(out=za[1:32, 32:1056], in_=z1[7][96:127, :])
            else:
                sync.dma_start(out=za[0:32, 32:1056], in_=z1[t - 1][96:128, :])

            zb = zb_p.tile([128, 1088], f32, tag="zb")
            V.memset(zb[:, 0:32], 0.0)
            V.memset(zb[:, 1056:1088], 0.0)
            sync.dma_start(out=zb[0:96, 32:1056], in_=z1[t][32:128, :])
            if t == 7:
                V.memset(zb[96:128, 32:1056], 0.0)
                sync.dma_start(out=zb[96:127, 32:1056], in_=z1[0][1:32, :])
            else:
                sync.dma_start(out=zb[96:128, 32:1056], in_=z1[t + 1][0:32, :])

            V.tensor_tensor(z2[t][:, :], z1[t][:, :], za[:, 0:1024], op=OP.add)
            V.tensor_tensor(z2[t][:, :], z2[t][:, :], zb[:, 64:1088], op=OP.add)
            V.tensor_tensor(z2[t][:, 1:32], z2[t][:, 1:32], za[:, 1024:1055],
                            op=OP.add)
            V.tensor_tensor(z2[t][:, 992:1023], z2[t][:, 992:1023], zb[:, 33:64],
                            op=OP.add)

        # ---------- per-half selection + softmax ----------
        abf = late.tile([128, 4096], bf16, tag="abf")
        atb = late.tile([128, 4096], bf16, tag="atb")

        def sel_tt(w2j, t):
            V.tensor_tensor(
                w2j[:].rearrange("c (a b) -> c a b", b=32),
                z2[t][:].rearrange("c (a b) -> c a b", b=32),
                m2v, op=OP.mult,
            )

        pid = nc.partition_id()
        for j in range(4):
            w2j = soft.tile([128, 1024], f32, tag="soft", name=f"w2_{j}")
            with tc.If(pid < 4) as cmp:
                sel_tt(w2j, j)
            with cmp.Else():
                sel_tt(w2j, 4 + j)
            rmax = sm.tile([128, 1], f32, tag="rmax")
            V.tensor_reduce(rmax[:, :], w2j[:, :], axis=mybir.AxisListType.X,
                            op=OP.max)
            nbias = sm.tile([128, 1], f32, tag="nbias")
            V.tensor_scalar_mul(nbias[:, :], rmax[:, :], -10.0)
            ex = expool.tile([128, 1024], f32, tag="ex")
            S.activation(ex[:, :], w2j[:, :], AF.Exp, bias=nbias[:, :], scale=10.0)
            rsum = sm.tile([128, 1], f32, tag="rsum")
            V.tensor_reduce(rsum[:, :], ex[:, :], axis=mybir.AxisListType.X,
                            op=OP.add)
            rrec = sm.tile([128, 1], f32, tag="rrec")
            V.reciprocal(rrec[:, :], rsum[:, :])
            V.tensor_scalar_mul(abf[:, 1024 * j:1024 * j + 1024], ex[:, :],
                                rrec[:, :])

        # ---------- A^T build: abf [q, p] -> atb [p, q_own], scaled by mm*0.25 ----------
        for pp in range(8):
            pst = pt.tile([128, 512], bf16, tag="pt")
            for j in range(4):
                nc.tensor.matmul(
                    pst[:, 128 * j:128 * j + 128],
                    abf[:, 1024 * j + 128 * pp:1024 * j + 128 * pp + 128],
                    ident[:, :],
                    is_transpose=True, start=(j == 0), stop=(j == 3),
                )
            V.tensor_scalar_mul(atb[:, 512 * pp:512 * pp + 512], pst[:, :],
                                mmq4[:, pp:pp + 1])

        # ---------- GEMM2 + scatter: out[c, 2i+kh-1, 2j+kw-1] += RW^T A ----------
        outv = outb[:].rearrange("c (r x) -> c r x", x=66)
        pov_shape = None
        for kh in range(4):
            for kw in range(4):
                pso = po.tile([128, 512], f32, tag="po")
                qp = qpl[(kh % 2, kw % 2, kw // 2)]
                kh2 = kh // 2
                for half in range(2):
                    pst = pt.tile([128, 512], bf16, tag="pt")
                    for u in range(4):
                        pp = 4 * half + u
                        bv = qp[:, (4 * pp + kh2) * 32:(4 * pp + kh2) * 32 + 128]
                        nc.tensor.matmul(
                            pst[:, 128 * u:128 * u + 128], bv, ident[:, :],
                            is_transpose=True, start=(u == 0), stop=(u == 3),
                        )
                    rwq = rwq_p.tile([128, 512], bf16, tag="rwq")
                    V.tensor_copy(rwq[:, :], pst[:, :])
                    for u in range(4):
                        pp = 4 * half + u
                        nc.tensor.matmul(
                            pso[:, :],
                            rwq[:, 128 * u:128 * u + 128],
                            atb[:, 512 * pp:512 * pp + 512],
                            start=(pp == 0), stop=(pp == 7),
                        )
                ov = outv[:, ds(kh, 16, 2), ds(kw, 32, 2)]
                psv = pso[:].rearrange("c (a b) -> c a b", b=32)
                V.tensor_tensor(ov, ov, psv, op=OP.add)

        sync.dma_start(out=out_d[:, :, :], in_=outb[:].rearrange(
            "c (r x) -> c r x", x=66))

    nc.compile()
    return nc


def _get_nc():
    global _NC
    if _NC is None:
        _NC = _build_nc()
    return _NC


def kernel(f, b, mask):
    from concourse.bass_utils import run_bass_kernel_spmd

    nc = _get_nc()
    f = np.ascontiguousarray(f, dtype=np.float32)
    b = np.ascontiguousarray(b, dtype=np.float32)
    mask = np.ascontiguousarray(mask, dtype=np.float32)

    in_maps = []
    for h in range(2):
        for s in range(4):
            in_maps.append({"fb": f[s], "bb": b[s], "mb": mask[s]})

    trace = os.environ.get("CA_TRACE", "0") == "1"
    res = run_bass_kernel_spmd(
        nc, in_maps, core_ids=list(range(8)), trace=trace,
    )
    if trace and res.exec_time_ns is not None:
        print(f"HW exec time: {res.exec_time_ns} ns")

    out = np.zeros((4, 128, 64, 64), dtype=np.float32)
    for s in range(4):
        band0 = res.results[s]["out"]        # h=0: rows rp 1..33 -> y 0..32
        band1 = res.results[4 + s]["out"]    # h=1: rows rp 0..32 -> y 31..63
        out[s][:, 0:33, :] += band0[:, 1:34, 1:65]
        out[s][:, 31:64, :] += band1[:, 0:33, 1:65]
    return out



# revision 24
# speedup vs baseline: 2.4610x; 2.4610x over previous
# Trainium2 Bass kernel for nn_ContextualAttention_5669356832394.
#
# Reference computation (per sample):
#   - downsample f,b,mask by 2 (nearest)
#   - w  = 3x3 patches of b_s (L=1024 patches), normalized
#   - Y[p, q] = <w_p, patch_q(f_s)>  (correlation GEMM, K=1152)
#   - two "fuse" convs: diagonal 3-tap sums over flattened (Lb, Lf) with a
#     digit-swap permutation between passes
#   - softmax over p (the 1024 b-patches) per f-site, scaled by mask patch mean
#   - deconv (conv_transpose stride 2, kernel 4) with raw 4x4 patches of b
#
# Sharding: 8 cores = 4 samples x 2 halves of the f-site grid (fi<16 / fi>=16).
# Each core computes the full Y (SPMD-identical GEMM1), fuse pass 1 for all 8
# q-tiles, then fuse pass 2 + softmax + deconv only for its own 4 q-tiles
# (selected with tc.If on partition_id). Host sums the 2-row overlap of the two
# output bands per sample.
#
# Fuse passes do their cross-partition shifts on the TensorEngine: identity
# matrices shifted by +-1 / +-32 columns used as matmul weights, accumulating
# the shifted terms (including cross-tile carries and the digit-swap wrap
# terms) into PSUM, with the free-axis +-1/+-32 shifts expressed as column
# offsets on zero-margined y/z1 tiles.  This replaces the baseline's
# SBUF->SBUF partition-shift DMAs (~16MB of 4KB descriptors serialized on
# one queue), which dominated its 553us runtime.

import os
import numpy as np

_NC = None


def _build_nc():
    import concourse.bass as bass
    import concourse.mybir as mybir
    import concourse.tile as tile
    from concourse import bacc
    from concourse.masks import make_identity
    from contextlib import ExitStack

    dt = mybir.dt
    AF = mybir.ActivationFunctionType
    OP = mybir.AluOpType
    ds = bass.ds

    nc = bacc.Bacc(
        "TRN2",
        target_bir_lowering=False,
        debug=False,
        num_devices=8,
        enable_partition_id=True,
    )

    fb = nc.dram_tensor("fb", [128, 64, 64], dt.float32, kind="ExternalInput").ap()
    bb = nc.dram_tensor("bb", [128, 64, 64], dt.float32, kind="ExternalInput").ap()
    mb = nc.dram_tensor("mb", [1, 64, 64], dt.float32, kind="ExternalInput").ap()
    out_d = nc.dram_tensor("out", [128, 36, 66], dt.float32, kind="ExternalOutput").ap()
    msm_d = nc.dram_tensor("msm_scr", [1, 1024], dt.float32, kind="Internal").ap()
    inv_d = nc.dram_tensor("inv_scr", [1, 1024], dt.float32, kind="Internal").ap()

    with tile.TileContext(nc) as tc, ExitStack() as ctx:
        sing = ctx.enter_context(tc.tile_pool(name="sing", bufs=1))
        scr = ctx.enter_context(tc.tile_pool(name="scr", bufs=2))
        sm = ctx.enter_context(tc.tile_pool(name="sm", bufs=4))
        rwq_p = ctx.enter_context(tc.tile_pool(name="rwq", bufs=2))

        f32, bf16, f32r = dt.float32, dt.bfloat16, dt.float32r

        # ---------- persistent SBUF tensors ----------
        m2 = sing.tile([128, 1088], f32, tag="m2")      # mask patch mean /9 (windowed)
        outb = sing.tile([128, 2376], f32, tag="outb")  # output band [c, 36*66]
        inv_rep = sing.tile([128, 1024], f32, tag="invrep")
        mmq4 = sing.tile([128, 8], f32, tag="mmq4")
        ones_c = sing.tile([128, 1], f32, tag="ones")
        ident = sing.tile([128, 128], bf16, tag="ident")
        identF = sing.tile([128, 128], f32, tag="identF")
        # shifted identities: S_*[k, i] = 1 iff k == i - s (matmul out[i] = in[i-s]).
        # All fuse weights are full 128x128 (f32r matmuls reject PE sub-tiles);
        # carry matrices are mostly-zero and accumulate zeros elsewhere.
        S_d1 = sing.tile([128, 128], f32r, tag="sd1")    # out[i] = in[i-1]
        S_u1 = sing.tile([128, 128], f32r, tag="su1")    # out[i] = in[i+1]
        S_d32 = sing.tile([128, 128], f32r, tag="sd32")  # out[i] = in[i-32]
        S_u32 = sing.tile([128, 128], f32r, tag="su32")  # out[i] = in[i+32]
        C_dn1 = sing.tile([128, 128], f32r, tag="cdn1")  # out[0] = in[127]
        C_up1 = sing.tile([128, 128], f32r, tag="cup1")  # out[127] = in[0]
        S_u96 = sing.tile([128, 128], f32r, tag="su96")  # out[i] = in[i+96], i<32
        S_d96 = sing.tile([128, 128], f32r, tag="sd96")  # out[i] = in[i-96], i>=96
        W_za = sing.tile([128, 128], f32r, tag="wza")    # out[i] = in[95+i], 1<=i<32
        W_zb = sing.tile([128, 128], f32r, tag="wzb")    # out[96+i] = in[1+i], i<31
        ident1 = sing.tile([1, 1], f32, tag="ident1")
        # pitch-32 window planes: every GEMM1 operand is a flat span.
        fq = [sing.tile([128, 1088], f32r, tag=f"fq{k}", name=f"fq{k}")
              for k in range(3)]  # fq[kw][c, r*32+u] = fs_pad[c, r, u+kw]
        bq = [sing.tile([128, 1088], f32r, tag=f"bq{k}", name=f"bq{k}")
              for k in range(3)]  # bq[kw][c, r*32+u] = bs_pad[c, r, u+kw]
        # qpl[(alpha, beta, kw2)][c, r*32+u] = b_pad2[c, 2r+alpha, 2(u+kw2)+beta]
        qpl = {}
        for al in range(2):
            for be in range(2):
                for kw2 in range(2):
                    qpl[(al, be, kw2)] = sing.tile(
                        [128, 1056], bf16, tag=f"q{al}{be}{kw2}",
                        name=f"q{al}{be}{kw2}")

        # y tiles: logical data in cols [1, 1025), zero margin at 0 and 1025
        y = [sing.tile([128, 1026], f32r, tag=f"y{t}", name=f"y{t}")
             for t in range(8)]
        # z1 tiles: logical data in cols [64, 1088), zero margins [0,64)+[1088,1152).
        # z1[t] (t>=1) aliases y[t-1]'s slot: y[t-1]'s last read is pass-1 tile
        # t's carry matmul, which precedes the z1[t] write in the dep chain.
        z1 = [sing.tile([128, 1152], f32r,
                        tag=("z1_0" if t == 0 else f"y{t - 1}"), name=f"z1{t}")
              for t in range(8)]

        sync = nc.sync
        gps = nc.gpsimd
        V = nc.vector
        S = nc.scalar

        early_cm = tc.tile_pool(name="early", bufs=1)
        early = early_cm.__enter__()
        fstage = early.tile([128, 2048], f32, tag="fstage")  # even rows of f
        bstage = early.tile([128, 4096], f32, tag="bstage")  # full b
        fs = early.tile([128, 1156], f32, tag="fs")     # padded f_s [c, 34*34]
        bs = early.tile([128, 1160], f32, tag="bs")     # padded b_s [c, 34*34]
        bfb = early.tile([128, 4356], bf16, tag="bfb")  # padded b [c, 66*66] bf16
        msr = scr.tile([128, 1156], f32, tag="scr", name="msr")  # bcast mask_s
        # packed [1, *] row vectors (each [1,1024]): norm2 | invw | invw2 | mmc
        pk = early.tile([1, 4096], f32, tag="pk")
        norm2 = pk[:, 0:1024]
        inv_w = pk[:, 1024:2048]
        inv_w2 = pk[:, 2048:3072]
        mmc = pk[:, 3072:4096]
        mrow = early.tile([32, 64], f32, tag="mrow")   # even mask rows on partitions
        msds = early.tile([32, 32], f32, tag="msds")   # downsampled mask rows

        # ---------- input loads (full tensors, downsample on-chip) ----------
        gps.memset(outb[:, :], 0.0)
        gps.memset(fs[:, :], 0.0)
        gps.memset(bs[:, :], 0.0)
        gps.memset(bfb[:, :], 0.0)
        gps.memset(msr[:, :], 0.0)

        fsr = fs[:].rearrange("c (h w) -> c h w", w=34)
        bsr = bs[:, 0:1156].rearrange("c (h w) -> c h w", w=34)
        bfr = bfb[:].rearrange("c (h w) -> c h w", w=66)
        msv = msr[:].rearrange("c (h w) -> c h w", w=34)

        fgv = fstage[:].rearrange("c (h w) -> c h w", w=64)
        bgv = bstage[:].rearrange("c (h w) -> c h w", w=64)
        sync.dma_start(out=fgv[:, :, :], in_=fb[:, ds(0, 32, 2), :])
        sync.dma_start(out=bgv[:, :, :], in_=bb[:, :, :])
        V.tensor_copy(fsr[:, 1:33, 1:33], fgv[:, :, ds(0, 32, 2)])
        V.tensor_copy(bsr[:, 1:33, 1:33], bgv[:, ds(0, 32, 2), ds(0, 32, 2)])
        S.copy(bfr[:, 1:65, 1:65], bgv[:, :, :])

        sync.dma_start(out=mrow[:, :], in_=mb[0, ds(0, 32, 2), :])
        V.tensor_copy(msds[:, :], mrow[:, ds(0, 32, 2)])
        msm_d32 = bass.AP(tensor=msm_d.tensor, offset=msm_d.offset,
                          ap=[[32, 32], [1, 32]])
        sync.dma_start(out=msm_d32, in_=msds[:, :])
        msm_bc = bass.AP(
            tensor=msm_d.tensor, offset=msm_d.offset,
            ap=[[0, 128], [32, 32], [1, 32]],
        )
        gps.dma_start(out=msv[:, 1:33, 1:33], in_=msm_bc)

        # ---------- identity + shift matrices ----------
        make_identity(nc, identF[:])
        V.tensor_copy(ident[:, :], identF[:, :])
        V.memset(ones_c[:, :], 1.0)
        V.memset(ident1[:, :], 1.0)
        # memsets on f32r tiles must go through an f32 bitcast view (walrus
        # rejects Memset with an f32r dtype)
        V.memset(S_d1.bitcast(f32)[:, :], 0.0)
        V.tensor_copy(S_d1[:, 1:128], identF[:, 0:127])
        V.memset(S_u1.bitcast(f32)[:, :], 0.0)
        V.tensor_copy(S_u1[:, 0:127], identF[:, 1:128])
        V.memset(S_d32.bitcast(f32)[:, :], 0.0)
        V.tensor_copy(S_d32[:, 32:128], identF[:, 0:96])
        V.memset(S_u32.bitcast(f32)[:, :], 0.0)
        V.tensor_copy(S_u32[:, 0:96], identF[:, 32:128])
        V.memset(C_dn1.bitcast(f32)[:, :], 0.0)
        V.tensor_copy(C_dn1[:, 0:1], identF[:, 127:128])
        V.memset(C_up1.bitcast(f32)[:, :], 0.0)
        V.tensor_copy(C_up1[:, 127:128], identF[:, 0:1])
        V.memset(S_u96.bitcast(f32)[:, :], 0.0)
        V.tensor_copy(S_u96[:, 0:32], identF[:, 96:128])
        V.memset(S_d96.bitcast(f32)[:, :], 0.0)
        V.tensor_copy(S_d96[:, 96:128], identF[:, 0:32])
        V.memset(W_za.bitcast(f32)[:, :], 0.0)
        V.tensor_copy(W_za[:, 1:32], identF[:, 96:127])
        V.memset(W_zb.bitcast(f32)[:, :], 0.0)
        V.tensor_copy(W_zb[:, 96:127], identF[:, 1:32])

        # y zero margins (z1 margins are set inside the pass-1 loop, after the
        # aliased y slot's last read)
        for t in range(8):
            V.memset(y[t].bitcast(f32)[:, 0:1], 0.0)
            V.memset(y[t].bitcast(f32)[:, 1025:1026], 0.0)

        # ---------- window planes ----------
        for kw in range(3):
            V.tensor_copy(fq[kw][:].rearrange("c (r u) -> c r u", u=32),
                          fsr[:, 0:34, kw:kw + 32])
            V.tensor_copy(bq[kw][:].rearrange("c (r u) -> c r u", u=32),
                          bsr[:, 0:34, kw:kw + 32])
        for al in range(2):
            for be in range(2):
                for kw2 in range(2):
                    S.copy(qpl[(al, be, kw2)][:].rearrange("c (r u) -> c r u", u=32),
                           bfr[:, ds(al, 33, 2), ds(2 * kw2 + be, 32, 2)])

        # ---------- mask patch means (x 1/9) ----------
        m1 = scr.tile([128, 1154], f32, tag="scr")
        V.tensor_tensor(m1[:, :], msr[:, 0:1154], msr[:, 1:1155], op=OP.add)
        V.tensor_tensor(m1[:, :], m1[:, :], msr[:, 2:1156], op=OP.add)
        V.tensor_tensor(m2[:, 0:1086], m1[:, 0:1086], m1[:, 34:1120], op=OP.add)
        V.tensor_tensor(m2[:, 0:1086], m2[:, 0:1086], m1[:, 68:1154], op=OP.add)
        V.tensor_scalar_mul(m2[:, 0:1086], m2[:, 0:1086], 1.0 / 9.0)
        m2v = m2[:].rearrange("c (a b) -> c a b", b=34)[:, :, 0:32]  # [128,32,32] mm

        pn_cm = tc.tile_pool(name="pn", bufs=1, space="PSUM")
        pn = pn_cm.__enter__()

        # mmq4[part, pp] = mm[128*pp + part] * 0.25  (folds deconv /4 + 2nd mask mult)
        V.tensor_copy(mmc[0:1].rearrange("c (a b) -> c a b", b=32),
                      m2v[0:1, :, :])
        psq = pn.tile([128, 8], f32, tag="pnq")
        for pp in range(8):
            nc.tensor.matmul(psq[:, pp:pp + 1], mmc[0:1, 128 * pp:128 * pp + 128],
                             ident1[:, :], is_transpose=True,
                             start=(pp == 0), stop=(pp == 7))
        V.tensor_scalar_mul(mmq4[:, :], psq[:, :], 0.25)

        # ---------- patch norms -> inv_w, inv_rep ----------
        sq = scr.tile([128, 1156], f32, tag="scr")
        S.activation(sq[:, :], bs[:, 0:1156], AF.Square)
        r1 = scr.tile([128, 1154], f32, tag="scr")
        V.tensor_tensor(r1[:, :], sq[:, 0:1154], sq[:, 1:1155], op=OP.add)
        V.tensor_tensor(r1[:, :], r1[:, :], sq[:, 2:1156], op=OP.add)
        r2 = scr.tile([128, 1088], f32, tag="scr")
        V.memset(r2[:, 1086:1088], 0.0)
        V.tensor_tensor(r2[:, 0:1086], r1[:, 0:1086], r1[:, 34:1120], op=OP.add)
        V.tensor_tensor(r2[:, 0:1086], r2[:, 0:1086], r1[:, 68:1154], op=OP.add)
        for n in range(4):
            psn = pn.tile([1, 272], f32, tag="pn")
            nc.tensor.matmul(
                psn[:, :], ones_c[:, :], r2[:, 272 * n:272 * n + 272],
                start=True, stop=True,
            )
            # norm2 = patch_sumsq + 1152e-4 (drop the 2 pitch-pad columns/row)
            psnv = psn[:].rearrange("c (a b) -> c a b", b=34)[:, :, 0:32]
            S.activation(norm2[0:1].rearrange("c (a b) -> c a b", b=32)[:, 8 * n:8 * n + 8, :],
                         psnv, AF.Copy, bias=0.1152)
        S.activation(inv_w[:, :], norm2[:, :], AF.Sqrt)
        V.reciprocal(inv_w[:, :], inv_w[:, :])
        # one Newton step: r' = r * (1.5 - 0.5 * x * r^2) cleans up Sqrt's ULPs
        V.tensor_tensor(inv_w2[:, :], inv_w[:, :], inv_w[:, :], op=OP.mult)
        V.tensor_tensor(inv_w2[:, :], inv_w2[:, :], norm2[:, :], op=OP.mult)
        S.activation(inv_w2[:, :], inv_w2[:, :], AF.Copy, bias=1.5, scale=-0.5)
        V.tensor_tensor(inv_w2[:, :], inv_w[:, :], inv_w2[:, :], op=OP.mult)
        sync.dma_start(out=inv_d[:, :], in_=inv_w2[:, :])
        inv_src = bass.AP(
            tensor=inv_d.tensor, offset=inv_d.offset,
            ap=[[0, 128], [1, 1024]],
        )
        gps.dma_start(out=inv_rep[:, :], in_=inv_src)

        pn_cm.__exit__(None, None, None)
        early_cm.__exit__(None, None, None)
        soft = ctx.enter_context(tc.tile_pool(name="soft", bufs=2))
        late = ctx.enter_context(tc.tile_pool(name="late", bufs=1))

        # ---------- GEMM1: Y^T[q, p] = sum_k F[k,q] B[k,p] * inv_w[p] ----------
        pj_cm = tc.tile_pool(name="pj", bufs=2, space="PSUM")
        pj = pj_cm.__enter__()
        p1_cm = tc.tile_pool(name="p1", bufs=2, space="PSUM")
        p1 = p1_cm.__enter__()
        psy_l = [pj.tile([128, 512], f32, tag="pj", name=f"psy{k}") for k in range(2)]
        ps1_l = [p1.tile([128, 1024], f32, tag="p1", name=f"ps1_{k}") for k in range(2)]

        for t in range(8):
            for n in range(2):
                psy = psy_l[(2 * t + n) % 2]
                i = 0
                for kh in range(3):
                    for kw in range(3):
                        foff = (4 * t + kh) * 32
                        boff = (16 * n + kh) * 32
                        nc.tensor.matmul(
                            psy[:, :], fq[kw][:, foff:foff + 128],
                            bq[kw][:, boff:boff + 512],
                            start=(i == 0), stop=(i == 8),
                        )
                        i += 1
                V.tensor_tensor(
                    y[t][:, 1 + 512 * n:513 + 512 * n], psy[:, :],
                    inv_rep[:, 512 * n:512 * n + 512], op=OP.mult,
                )

        # ---------- fuse pass 1 on PE: Z1[q,p] = Y[q-1,p-1] + Y[q,p] + Y[q+1,p+1]
        for t in range(8):
            ps1 = ps1_l[t % 2]
            for c in range(2):
                c0 = 512 * c
                mms = [(S_d1, y[t][:, c0:c0 + 512])]
                if t > 0:
                    mms.append((C_dn1, y[t - 1][:, c0:c0 + 512]))
                if t < 7:
                    mms.append((C_up1, y[t + 1][:, 2 + c0:514 + c0]))
                mms.append((S_u1, y[t][:, 2 + c0:514 + c0]))
                for i, (lv, rv) in enumerate(mms):
                    nc.tensor.matmul(ps1[:, c0:c0 + 512], lv[:, :], rv,
                                     start=(i == 0), stop=(i == len(mms) - 1))
            V.tensor_tensor(z1[t][:, 64:1088], y[t][:, 1:1025], ps1[:, 0:1024],
                            op=OP.add)
            V.memset(z1[t].bitcast(f32)[:, 0:64], 0.0)
            V.memset(z1[t].bitcast(f32)[:, 1088:1152], 0.0)

        p1_cm.__exit__(None, None, None)
        pj_cm.__exit__(None, None, None)

        # ---------- fuse pass 2 (digit-swapped) + softmax, own half only ------
        p2_cm = tc.tile_pool(name="p2", bufs=2, space="PSUM")
        p2 = p2_cm.__enter__()
        ps2_l = [p2.tile([128, 1088], f32, tag="p2", name=f"ps2_{k}") for k in range(2)]
        abf = late.tile([128, 4096], bf16, tag="abf")
        atb = late.tile([128, 4096], bf16, tag="atb")
        w2_l = [soft.tile([128, 1024], f32, tag=f"w2_{t}", name=f"w2_{t}")
                for t in range(8)]
        rmax_l = [sm.tile([128, 1], f32, tag="rmax", name=f"rmax{j}") for j in range(4)]
        nbias_l = [sm.tile([128, 1], f32, tag="nbias", name=f"nbias{j}") for j in range(4)]
        rsum_l = [sm.tile([128, 1], f32, tag="rsum", name=f"rsum{j}") for j in range(4)]
        rrec_l = [sm.tile([128, 1], f32, tag="rrec", name=f"rrec{j}") for j in range(4)]

        for lt in range(8):
            ps2 = ps2_l[lt % 2]
            w2j = w2_l[lt]
            for (c0, cw) in ((0, 512), (512, 512), (1024, 64)):
                mms = [(S_d32, z1[lt][:, c0:c0 + cw])]
                if lt > 0:
                    mms.append((S_u96, z1[lt - 1][:, c0:c0 + cw]))
                else:
                    mms.append((W_za, z1[7][:, c0:c0 + cw]))
                if lt < 7:
                    mms.append((S_d96, z1[lt + 1][:, 64 + c0:64 + c0 + cw]))
                else:
                    mms.append((W_zb, z1[0][:, 64 + c0:64 + c0 + cw]))
                mms.append((S_u32, z1[lt][:, 64 + c0:64 + c0 + cw]))
                for i, (lv, rv) in enumerate(mms):
                    nc.tensor.matmul(ps2[:, c0:c0 + cw], lv[:, :], rv,
                                     start=(i == 0), stop=(i == len(mms) - 1))
            # w2 = (z1 + shifted terms) * mm
            V.tensor_tensor(w2j[:, 0:1024], z1[lt][:, 64:1088],
                            ps2[:, 32:1056], op=OP.add)
            V.tensor_tensor(w2j[:, 1:32], w2j[:, 1:32], ps2[:, 1056:1087],
                            op=OP.add)
            V.tensor_tensor(w2j[:, 992:1023], w2j[:, 992:1023], ps2[:, 1:32],
                            op=OP.add)
            V.tensor_tensor(
                w2j[:].rearrange("c (a b) -> c a b", b=32),
                w2j[:].rearrange("c (a b) -> c a b", b=32),
                m2v, op=OP.mult,
            )

        # ---------- softmax over p (free axis), own half only ----------
        pid = nc.partition_id()

        def softmax_j(j, lt):
            rmax, nbias, rsum, rrec = rmax_l[j], nbias_l[j], rsum_l[j], rrec_l[j]
            w2j = w2_l[lt]
            V.tensor_reduce(rmax[:, :], w2j[:, :],
                            axis=mybir.AxisListType.X, op=OP.max)
            V.tensor_scalar_mul(nbias[:, :], rmax[:, :], -10.0)
            S.activation(w2j[:, :], w2j[:, :], AF.Exp, bias=nbias[:, :],
                         scale=10.0)
            V.tensor_reduce(rsum[:, :], w2j[:, :], axis=mybir.AxisListType.X,
                            op=OP.add)
            V.reciprocal(rrec[:, :], rsum[:, :])
            V.tensor_scalar_mul(abf[:, 1024 * j:1024 * j + 1024], w2j[:, :],
                                rrec[:, :])

        for j in range(4):
            with tc.If(pid < 4) as cmp:
                softmax_j(j, j)
            with cmp.Else():
                softmax_j(j, 4 + j)

        p2_cm.__exit__(None, None, None)

        pt_cm = tc.tile_pool(name="pt", bufs=2, space="PSUM")
        pt = pt_cm.__enter__()
        po_cm = tc.tile_pool(name="po", bufs=2, space="PSUM")
        po = po_cm.__enter__()

        # ---------- A^T build: abf [q, p] -> atb [p, q_own], scaled by mm*0.25 ----
        for pp in range(8):
            pst = pt.tile([128, 512], bf16, tag="pt")
            for j in range(4):
                nc.tensor.matmul(
                    pst[:, 128 * j:128 * j + 128],
                    abf[:, 1024 * j + 128 * pp:1024 * j + 128 * pp + 128],
                    ident[:, :],
                    is_transpose=True, start=(j == 0), stop=(j == 3),
                )
            V.tensor_scalar_mul(atb[:, 512 * pp:512 * pp + 512], pst[:, :],
                                mmq4[:, pp:pp + 1])

        # ---------- GEMM2 + scatter: out[c, 2i+kh-1, 2j+kw-1] += RW^T A ----------
        outv = outb[:].rearrange("c (r x) -> c r x", x=66)
        for kh in range(4):
            for kw in range(4):
                pso = po.tile([128, 512], f32, tag="po")
                qp = qpl[(kh % 2, kw % 2, kw // 2)]
                kh2 = kh // 2
                for hf in range(2):
                    pst = pt.tile([128, 512], bf16, tag="pt")
                    for u in range(4):
                        pp = 4 * hf + u
                        bv = qp[:, (4 * pp + kh2) * 32:(4 * pp + kh2) * 32 + 128]
                        nc.tensor.matmul(
                            pst[:, 128 * u:128 * u + 128], bv, ident[:, :],
                            is_transpose=True, start=(u == 0), stop=(u == 3),
                        )
                    rwq = rwq_p.tile([128, 512], bf16, tag="rwq")
                    V.tensor_copy(rwq[:, :], pst[:, :])
                    for u in range(4):
                        pp = 4 * hf + u
                        nc.tensor.matmul(
                            pso[:, :],
                            rwq[:, 128 * u:128 * u + 128],
                            atb[:, 512 * pp:512 * pp + 512],
                            start=(pp == 0), stop=(pp == 7),
                        )
                ov = outv[:, ds(kh, 16, 2), ds(kw, 32, 2)]
                psv = pso[:].rearrange("c (a b) -> c a b", b=32)
                V.tensor_tensor(ov, ov, psv, op=OP.add)

        po_cm.__exit__(None, None, None)
        pt_cm.__exit__(None, None, None)

        for rc in range(4):
            sync.dma_start(out=out_d[:, 9 * rc:9 * rc + 9, :],
                           in_=outv[:, 9 * rc:9 * rc + 9, :])

    nc.compile()
    return nc


def _get_nc():
    global _NC
    if _NC is None:
        _NC = _build_nc()
    return _NC


def kernel(f, b, mask):
    from concourse.bass_utils import run_bass_kernel_spmd

    nc = _get_nc()
    f = np.ascontiguousarray(f, dtype=np.float32)
    b = np.ascontiguousarray(b, dtype=np.float32)
    mask = np.ascontiguousarray(mask, dtype=np.float32)

    in_maps = []
    for h in range(2):
        for s in range(4):
            in_maps.append({"fb": f[s], "bb": b[s], "mb": mask[s]})

    trace = os.environ.get("CA_TRACE", "0") == "1"
    res = run_bass_kernel_spmd(
        nc, in_maps, core_ids=list(range(8)), trace=trace,
    )
    if trace and res.exec_time_ns is not None:
        print(f"HW exec time: {res.exec_time_ns} ns")

    out = np.zeros((4, 128, 64, 64), dtype=np.float32)
    for s in range(4):
        band0 = res.results[s]["out"]        # h=0: rows rp 1..33 -> y 0..32
        band1 = res.results[4 + s]["out"]    # h=1: rows rp 0..32 -> y 31..63
        out[s][:, 0:33, :] += band0[:, 1:34, 1:65]
        out[s][:, 31:64, :] += band1[:, 0:33, 1:65]
    return out
